# revision 1
# baseline (speedup 1.0000x reference)
"""XL-BOMD rank-4 Krylov propagation (EnergyXL) on 8 TRN2 NeuronCores.

Data-parallel over molecules: 512 mols -> 64 per core. Per molecule
(N=192, rank=4):
  dDS = D - P ; v0 = dDS/||dDS||
  for k in 0..3:  T = v_k R ; W_k = R T - v_k   (PE sandwiches, fp32)
                  v_{k+1} = GS-orthonormalize(W_k vs v_0..v_k)
  O[IJ] = <W_I,W_J>, c[J] = <W_J,dDS>  (Frobenius ips, DVE TTR)
  y = O^-1 c (batched symmetric Gauss elim over mol-partition layout)
  out = -sum_I y_I V_I

Matrices live in SBUF as hi [128,192] (rows 0:128) + lo [64,192]
(rows 128:192) fp32 tiles.  Inner products use fused
tensor_tensor_reduce with hi->lo accumulator chaining; cross-partition
sums + per-mol scalar broadcast via ones-matmul on the PE.
"""

import os
import sys

sys.path.insert(0, "/opt/trn_rl_repo")

import numpy as np

import concourse.bass as bass
import concourse.bacc as bacc
import concourse.tile as tile
from concourse import mybir
from concourse.bass_utils import run_bass_kernel_spmd

F32 = mybir.dt.float32
ALU = mybir.AluOpType
ACTF = mybir.ActivationFunctionType

NMOL, N, RANK = 512, 192, 4
NCORES = 8
MPC = NMOL // NCORES  # 64 molecules per core
HI, LO = 128, 64
BLK = 8  # molecules per solve block

# partials column map (per molecule, [128, 48] tile); every quantity is a
# (hi, lo) column pair summed post-broadcast (strided pair-add).
# Layout is rank-contiguous so ONE ones-matmul per rank broadcasts
# O_kk + c_k + GS coefs together:
#   col 0,1: ||dDS||^2
#   rank-k block at RBASE[k]: O(k,0..k) pairs, c_k pair, GS(k+1, 0..k) pairs
COL_N0 = 0
RBASE = [2, 8, 18, 32]          # rank block bases; sizes 6,10,14,10
NPART = 48
# post-gather pair-added column indices (gather covers cols 2..41 -> 20 pairs)
GIX = {"O00": 0, "c0": 1, "G10": 2, "O10": 3, "O11": 4, "c1": 5, "G20": 6,
       "G21": 7, "O20": 8, "O21": 9, "O22": 10, "c2": 11, "G30": 12,
       "G31": 13, "G32": 14, "O30": 15, "O31": 16, "O32": 17, "O33": 18,
       "c3": 19}


def _o_col(k, j):
    assert j <= k
    return RBASE[k] + 2 * j


def _c_col(k):
    return RBASE[k] + 2 * (k + 1)


def _gs_col(kk, j):
    # coef <w_k, v_j> for v_kk (kk = k+1), stored in rank-k block
    k = kk - 1
    return RBASE[k] + 2 * (k + 1) + 2 + 2 * j


def build_core_kernel(n_mols=MPC):
    nc = bacc.Bacc(None, target_bir_lowering=False, enable_partition_id=False)
    D = nc.dram_tensor("D", [n_mols, N, N], F32, kind="ExternalInput")
    P = nc.dram_tensor("P", [n_mols, N, N], F32, kind="ExternalInput")
    R = nc.dram_tensor("Rm", [n_mols, N, N], F32, kind="ExternalInput")
    OUT = nc.dram_tensor("OUT", [n_mols, N, N], F32, kind="ExternalOutput")

    with tile.TileContext(nc) as tc:
        _body(nc, tc, D, P, R, OUT)
    nc.finalize()
    return nc


def _body(nc, tc, D, P, R, OUT):
    import contextlib

    ctx = contextlib.ExitStack()
    with ctx:
        consts = ctx.enter_context(tc.tile_pool(name="consts", bufs=1))
        persist = ctx.enter_context(tc.tile_pool(name="persist", bufs=11))
        fast = ctx.enter_context(tc.tile_pool(name="fast", bufs=6))
        temps = ctx.enter_context(tc.tile_pool(name="temps", bufs=8))
        scal = ctx.enter_context(tc.tile_pool(name="scal", bufs=16))
        blkp = ctx.enter_context(tc.tile_pool(name="blkp", bufs=2))
        ps_mm = ctx.enter_context(tc.tile_pool(name="ps_mm", bufs=2, space="PSUM"))
        ps_sm = ctx.enter_context(tc.tile_pool(name="ps_sm", bufs=2, space="PSUM"))
        ps_g = ctx.enter_context(tc.tile_pool(name="ps_g", bufs=2, space="PSUM"))

        # --- constants ---
        ones = consts.tile([HI, HI], F32)      # all-ones for partition-sum / bcast matmuls
        nc.vector.memset(ones, 1.0)
        sel = consts.tile([HI, 2 * LO - 1], F32)  # windowed one-hot column selector
        nc.vector.memset(sel, 0.0)
        nc.vector.memset(sel[:, LO - 1 : LO], 1.0)
        id8 = consts.tile([BLK, BLK], F32)     # identity for y row-masking
        idt = consts.tile([BLK, BLK], mybir.dt.int32)
        nc.gpsimd.iota(idt, pattern=[[-1, BLK]], base=0, channel_multiplier=1)
        nc.vector.tensor_scalar(out=id8, in0=idt, scalar1=0, scalar2=None,
                                op0=ALU.is_equal)

        n_mols = D.shape[0]
        for b in range(n_mols // BLK):
            mols = list(range(b * BLK, (b + 1) * BLK))
            blk_state = []
            for m in mols:
                st = _mol_pipeline(nc, tc, D, P, R, m, persist, fast, temps, scal,
                                   ps_mm, ps_sm, ones)
                blk_state.append(st)
            _block_tail(nc, tc, OUT, b, mols, blk_state, consts, fast, temps, scal,
                        blkp, ps_g, ps_sm, ones, sel, id8)


def _ip(nc, partials, col2, a, b_, scr, scr2, mult_eng="dve", red="act"):
    """<A,B> Frobenius: elementwise mult then free-dim reduce into col pair."""
    a_hi, a_lo = a
    b_hi, b_lo = b_
    scr_hi, scr_lo = scr
    me = nc.vector if mult_eng == "dve" else nc.gpsimd
    me.tensor_mul(scr_hi, a_hi, b_hi)
    me.tensor_mul(scr_lo, a_lo, b_lo)
    if red == "act":
        nc.scalar.activation(out=scr_hi, in_=scr_hi, func=ACTF.Copy,
                             accum_out=partials[:, col2 : col2 + 1])
        nc.scalar.activation(out=scr_lo, in_=scr_lo, func=ACTF.Copy,
                             accum_out=partials[:LO, col2 + 1 : col2 + 2])
    else:
        nc.vector.tensor_reduce(out=partials[:, col2 : col2 + 1], in_=scr_hi,
                                axis=mybir.AxisListType.X, op=ALU.add)
        nc.vector.tensor_reduce(out=partials[:LO, col2 + 1 : col2 + 2], in_=scr_lo,
                                axis=mybir.AxisListType.X, op=ALU.add)


def _norm_sq(nc, partials, col2, x, scr_hi, scr_lo):
    """||X||^2 on ACT: square+accumulate, hi/lo to cols col2, col2+1."""
    x_hi, x_lo = x
    nc.scalar.activation(out=scr_hi, in_=x_hi, func=ACTF.Square,
                         accum_out=partials[:, col2 : col2 + 1])
    nc.scalar.activation(out=scr_lo, in_=x_lo, func=ACTF.Square,
                         accum_out=partials[:LO, col2 + 1 : col2 + 2])


def _bcast(nc, ps_sm, ones, partials, col, ncols):
    """ones-matmul: col sums of partials[:, col:col+ncols] broadcast to 128 partitions."""
    bc = ps_sm.tile([HI, ncols], F32, tag="bc")
    nc.tensor.matmul(bc, lhsT=ones, rhs=partials[:, col : col + ncols],
                     start=True, stop=True)
    return bc


def _inv_norm(nc, scal, ps_sm, ones, partials, col2, tag):
    """1/sqrt(hi_col + lo_col) as a [128,1] SBUF tile."""
    bc = _bcast(nc, ps_sm, ones, partials, col2, 2)
    s = scal.tile([HI, 5], F32, tag=tag)
    nc.scalar.copy(s[:, 0:2], bc)
    nc.vector.tensor_add(s[:, 2:3], s[:, 0:1], s[:, 1:2])
    nc.scalar.sqrt(s[:, 3:4], s[:, 2:3])
    nc.vector.reciprocal(s[:, 4:5], s[:, 3:4])
    return s[:, 4:5]


def _sandwich(nc, ps_mm, out_sb, lhsT, rhs):
    """out = lhsT^T @ rhs for 192x192 operands in hi/lo tiles -> PSUM pair."""
    l_hi, l_lo = lhsT
    r_hi, r_lo = rhs
    o_hi = ps_mm.tile([HI, N], F32, tag="mm_hi")
    o_lo = ps_mm.tile([LO, N], F32, tag="mm_lo")
    nc.tensor.matmul(o_hi, lhsT=l_hi[:, 0:HI], rhs=r_hi, start=True, stop=False)
    nc.tensor.matmul(o_hi, lhsT=l_lo[:, 0:HI], rhs=r_lo, start=False, stop=True)
    nc.tensor.matmul(o_lo, lhsT=l_hi[:, HI:N], rhs=r_hi, start=True, stop=False)
    nc.tensor.matmul(o_lo, lhsT=l_lo[:, HI:N], rhs=r_lo, start=False, stop=True)
    return o_hi, o_lo


def _mol_pipeline(nc, tc, D, P, R, m, persist, fast, temps, scal, ps_mm, ps_sm, ones):
    """Emit one molecule's Krylov chain. Returns state dict for the block tail."""
    # --- load R, D, P ---
    r_hi = fast.tile([HI, N], F32, tag="r_hi")
    r_lo = fast.tile([LO, N], F32, tag="r_lo")
    nc.sync.dma_start(out=r_hi, in_=R[m, 0:HI, :])
    nc.sync.dma_start(out=r_lo, in_=R[m, HI:N, :])

    dds_hi = persist.tile([HI, N], F32, tag="dds_hi")
    dds_lo = persist.tile([LO, N], F32, tag="dds_lo")
    nc.sync.dma_start(out=dds_hi, in_=D[m, 0:HI, :])
    nc.sync.dma_start(out=dds_lo, in_=D[m, HI:N, :])
    p_hi = temps.tile([HI, N], F32, tag="p_hi")
    p_lo = temps.tile([LO, N], F32, tag="p_lo")
    nc.sync.dma_start(out=p_hi, in_=P[m, 0:HI, :])
    nc.sync.dma_start(out=p_lo, in_=P[m, HI:N, :])
    nc.gpsimd.tensor_sub(dds_hi, dds_hi, p_hi)
    nc.gpsimd.tensor_sub(dds_lo, dds_lo, p_lo)
    dds = (dds_hi, dds_lo)

    partials = scal.tile([HI, NPART], F32, tag="partials")
    nc.vector.memset(partials, 0.0)

    scr_hi = temps.tile([HI, N], F32, tag="scr_hi")
    scr_lo = temps.tile([LO, N], F32, tag="scr_lo")
    scr = (scr_hi, scr_lo)
    scr2 = None

    # --- v0 = dDS / ||dDS|| ---
    _norm_sq(nc, partials, COL_N0, dds, scr_hi, scr_lo)
    inv0 = _inv_norm(nc, scal, ps_sm, ones, partials, COL_N0, tag="nrm0")
    v_tiles = []
    v0_hi = persist.tile([HI, N], F32, tag="v0_hi")
    v0_lo = persist.tile([LO, N], F32, tag="v0_lo")
    nc.scalar.mul(v0_hi, dds_hi, inv0)
    nc.scalar.mul(v0_lo, dds_lo, inv0[:LO, :])
    v_tiles.append((v0_hi, v0_lo))

    w_tiles = []
    for k in range(RANK):
        vk = v_tiles[k]
        # T = v_k R
        t_ps = _sandwich(nc, ps_mm, None, vk, (r_hi, r_lo))
        t_hi = temps.tile([HI, N], F32, tag="t_hi")
        t_lo = temps.tile([LO, N], F32, tag="t_lo")
        nc.scalar.copy(t_hi, t_ps[0])
        nc.scalar.copy(t_lo, t_ps[1])
        # W_k = R T - v_k
        w_ps = _sandwich(nc, ps_mm, None, (r_hi, r_lo), (t_hi, t_lo))
        w_hi = fast.tile([HI, N], F32, tag=f"w{k}_hi")
        w_lo = fast.tile([LO, N], F32, tag=f"w{k}_lo")
        nc.vector.tensor_sub(w_hi, w_ps[0], vk[0])
        nc.vector.tensor_sub(w_lo, w_ps[1], vk[1])
        wk = (w_hi, w_lo)
        w_tiles.append(wk)

        # O row k and c_k  (off the critical chain)
        for j in range(k + 1):
            _ip(nc, partials, _o_col(k, j), w_tiles[j], wk, scr, scr2,
                mult_eng="dve", red="act")
        _ip(nc, partials, _c_col(k), wk, dds, scr, scr2,
            mult_eng="gpsimd", red="dve")

        # Gram-Schmidt -> v_{k+1}; ||u||^2 = O_kk - sum c_j^2 (no 2nd pass)
        if k < RANK - 1:
            kk = k + 1
            for j in range(kk):
                _ip(nc, partials, _gs_col(kk, j), wk, v_tiles[j], scr, scr2,
                    mult_eng="dve", red="act")
            # one broadcast for O_kk + c_k + GS coefs: cols RBASE[k]+2k ..
            b0 = RBASE[k] + 2 * k
            nb = 4 + 2 * kk
            bc = _bcast(nc, ps_sm, ones, partials, b0, nb)
            s = scal.tile([HI, nb + 2 * kk + 6], F32, tag="gs_s")
            nc.scalar.copy(s[:, 0:nb], bc)
            x = nb
            coefs = s[:, x : x + kk]
            # pair-sum GS cols (offset 4 within block: after O_kk, c_k pairs)
            nc.vector.tensor_add(coefs, s[:, 4 : 4 + 2 * kk : 2],
                                 s[:, 5 : 4 + 2 * kk : 2])
            okk = s[:, x + kk : x + kk + 1]
            # ||u||^2 = (O_kk_hi + O_kk_lo) - sum_j coef_j^2
            u2 = s[:, x + kk + 1 : x + kk + 2]
            sq = s[:, x + kk + 2 : x + kk + 2 + kk]
            nc.vector.tensor_mul(sq, coefs, coefs)
            nc.vector.tensor_reduce(out=u2, in_=sq, axis=mybir.AxisListType.X,
                                    op=ALU.add)
            nc.vector.tensor_add(okk, s[:, 0:1], s[:, 1:2])
            u2b = s[:, x + 2 * kk + 2 : x + 2 * kk + 3]
            nc.vector.tensor_sub(u2b, okk, u2)
            sqr = s[:, x + 2 * kk + 3 : x + 2 * kk + 4]
            nc.scalar.sqrt(sqr, u2b)
            invn = s[:, x + 2 * kk + 4 : x + 2 * kk + 5]
            nc.vector.reciprocal(invn, sqr)

            u_hi = temps.tile([HI, N], F32, tag="u_hi")
            u_lo = temps.tile([LO, N], F32, tag="u_lo")
            for j in range(kk):
                ax_hi = temps.tile([HI, N], F32, tag="ax_hi")
                ax_lo = temps.tile([LO, N], F32, tag="ax_lo")
                nc.scalar.mul(ax_hi, v_tiles[j][0], coefs[:, j : j + 1])
                nc.scalar.mul(ax_lo, v_tiles[j][1], coefs[:LO, j : j + 1])
                src = wk if j == 0 else (u_hi, u_lo)
                nc.gpsimd.tensor_sub(u_hi, src[0], ax_hi)
                nc.gpsimd.tensor_sub(u_lo, src[1], ax_lo)
            vn_hi = persist.tile([HI, N], F32, tag=f"v{kk}_hi")
            vn_lo = persist.tile([LO, N], F32, tag=f"v{kk}_lo")
            nc.scalar.mul(vn_hi, u_hi, invn)
            nc.scalar.mul(vn_lo, u_lo, invn[:LO, :])
            v_tiles.append((vn_hi, vn_lo))

    return {"partials": partials, "v": v_tiles}


def _solve_sym4(nc, g, s):
    """Batched symmetric 4x4 solve on [BLK,1] column APs.

    g: [BLK, 14] tile, cols 0..9 = O (00,10,11,20,21,22,30,31,32,33),
    cols 10..13 = rhs c.  s: [BLK, 16] scratch.  Returns y col APs (in s cols 0..3).
    Mirrors _solve_sym4_np below; keep in sync.
    """
    def col(t, i):
        return t[:, i : i + 1]

    ox = [GIX[q] for q in ("O00", "O10", "O11", "O20", "O21", "O22",
                           "O30", "O31", "O32", "O33")]
    a, bb, e, c, f, h, d, gg, i_, jj = (col(g, i) for i in ox)
    r0, r1, r2, r3 = (col(g, GIX[f"c{i}"]) for i in range(4))
    p0, p1, p2, p3 = (col(s, 4 + i) for i in range(4))
    l1, l2, l3 = (col(s, 8 + i) for i in range(3))
    t0, t1 = col(s, 11), col(s, 12)
    y0, y1, y2, y3 = (col(s, i) for i in range(4))

    mul = nc.vector.tensor_mul
    sub = nc.vector.tensor_sub
    rec = nc.vector.reciprocal

    def upd(x, l, src):  # x -= l*src
        mul(t0, l, src)
        sub(x, x, t0)

    rec(p0, a)
    mul(l1, bb, p0); mul(l2, c, p0); mul(l3, d, p0)
    upd(e, l1, bb); upd(f, l2, bb); upd(gg, l3, bb)
    upd(h, l2, c); upd(i_, l3, c); upd(jj, l3, d)
    upd(r1, l1, r0); upd(r2, l2, r0); upd(r3, l3, r0)

    rec(p1, e)
    mul(l2, f, p1); mul(l3, gg, p1)
    upd(h, l2, f); upd(i_, l3, f); upd(jj, l3, gg)
    upd(r2, l2, r1); upd(r3, l3, r1)

    rec(p2, h)
    mul(l3, i_, p2)
    upd(jj, l3, i_); upd(r3, l3, r2)

    rec(p3, jj)
    mul(y3, r3, p3)
    # back-substitution
    upd(r2, i_, y3); mul(y2, r2, p2)
    upd(r1, f, y2); upd(r1, gg, y3); mul(y1, r1, p1)
    upd(r0, bb, y1); upd(r0, c, y2); upd(r0, d, y3); mul(y0, r0, p0)
    return [y0, y1, y2, y3]


def _solve_sym4_np(G):
    """NumPy mirror of _solve_sym4 for verification. G: [n, 14] -> y [n, 4]."""
    G = G.copy()
    cols = [G[:, i : i + 1] for i in range(14)]
    a, bb, e, c, f, h, d, gg, i_, jj = cols[:10]
    r0, r1, r2, r3 = cols[10:]
    p0 = 1.0 / a
    l1, l2, l3 = bb * p0, c * p0, d * p0
    e = e - l1 * bb; f = f - l2 * bb; gg = gg - l3 * bb
    h = h - l2 * c; i_ = i_ - l3 * c; jj = jj - l3 * d
    r1 = r1 - l1 * r0; r2 = r2 - l2 * r0; r3 = r3 - l3 * r0
    p1 = 1.0 / e
    l2, l3 = f * p1, gg * p1
    h = h - l2 * f; i_ = i_ - l3 * f; jj = jj - l3 * gg
    r2 = r2 - l2 * r1; r3 = r3 - l3 * r1
    p2 = 1.0 / h
    l3 = i_ * p2
    jj = jj - l3 * i_; r3 = r3 - l3 * r2
    p3 = 1.0 / jj
    y3 = r3 * p3
    r2 = r2 - i_ * y3; y2 = r2 * p2
    r1 = r1 - f * y2; r1 = r1 - gg * y3; y1 = r1 * p1
    r0 = r0 - bb * y1; r0 = r0 - c * y2; r0 = r0 - d * y3; y0 = r0 * p0
    return np.concatenate([y0, y1, y2, y3], axis=1)


def _block_tail(nc, tc, OUT, b, mols, blk_state, consts, fast, temps, scal, blkp,
                ps_g, ps_sm, ones, sel, id8):
    # gather each mol's 14 O/c sums into [BLK, 14] via selector matmuls
    gath = ps_g.tile([BLK, 40], F32, tag="gath")
    for j, st in enumerate(blk_state):
        nc.tensor.matmul(gath, lhsT=sel[:, LO - 1 - j : LO - 1 - j + BLK],
                         rhs=st["partials"][:, 2:42],
                         start=(j == 0), stop=(j == len(blk_state) - 1))
    g_pair = blkp.tile([BLK, 40], F32, tag="g_pair")
    nc.scalar.copy(g_pair, gath)
    g_sb = blkp.tile([BLK, 20], F32, tag="g_sb")
    nc.vector.tensor_add(g_sb, g_pair[:, 0:40:2], g_pair[:, 1:40:2])
    s_sb = blkp.tile([BLK, 16], F32, tag="s_sb")
    ys = _solve_sym4(nc, g_sb, s_sb)
    y_sb = blkp.tile([BLK, RANK], F32, tag="y_sb")
    for i in range(RANK):
        nc.vector.tensor_copy(y_sb[:, i : i + 1], ys[i])

    for j, (m, st) in enumerate(zip(mols, blk_state)):
        ymask = scal.tile([BLK, RANK], F32, tag="ymask")
        nc.vector.tensor_scalar(out=ymask, in0=y_sb, scalar1=id8[:, j : j + 1],
                                scalar2=None, op0=ALU.mult)
        ybc = ps_sm.tile([HI, RANK], F32, tag="bc")
        nc.tensor.matmul(ybc, lhsT=ones[0:BLK, :], rhs=ymask, start=True, stop=True)
        yb = scal.tile([HI, RANK], F32, tag="yb")
        nc.scalar.copy(yb, ybc)

        acc_hi = fast.tile([HI, N], F32, tag="acc_hi")
        acc_lo = fast.tile([LO, N], F32, tag="acc_lo")
        v = st["v"]
        nc.vector.tensor_scalar(out=acc_hi, in0=v[0][0], scalar1=yb[:, 0:1],
                                scalar2=-1.0, op0=ALU.mult, op1=ALU.mult)
        nc.vector.tensor_scalar(out=acc_lo, in0=v[0][1], scalar1=yb[:LO, 0:1],
                                scalar2=-1.0, op0=ALU.mult, op1=ALU.mult)
        for i in range(1, RANK):
            ax_hi = temps.tile([HI, N], F32, tag="ax_hi")
            ax_lo = temps.tile([LO, N], F32, tag="ax_lo")
            nc.vector.tensor_scalar(out=ax_hi, in0=v[i][0], scalar1=yb[:, i : i + 1],
                                    scalar2=None, op0=ALU.mult)
            nc.vector.tensor_scalar(out=ax_lo, in0=v[i][1], scalar1=yb[:LO, i : i + 1],
                                    scalar2=None, op0=ALU.mult)
            nc.gpsimd.tensor_sub(acc_hi, acc_hi, ax_hi)
            nc.gpsimd.tensor_sub(acc_lo, acc_lo, ax_lo)
        nc.sync.dma_start(out=OUT[m, 0:HI, :], in_=acc_hi)
        nc.sync.dma_start(out=OUT[m, HI:N, :], in_=acc_lo)


_NC_CACHE = None


def _get_nc():
    global _NC_CACHE
    if _NC_CACHE is None:
        _NC_CACHE = build_core_kernel()
    return _NC_CACHE


def kernel(D, P, R, max_rank=4, _trace=False):
    D = np.ascontiguousarray(D, dtype=np.float32)
    P = np.ascontiguousarray(P, dtype=np.float32)
    R = np.ascontiguousarray(R, dtype=np.float32)
    nc = _get_nc()
    in_maps = []
    for i in range(NCORES):
        sl = slice(i * MPC, (i + 1) * MPC)
        in_maps.append({"D": D[sl], "P": P[sl], "Rm": R[sl]})
    res = run_bass_kernel_spmd(nc, in_maps, core_ids=list(range(NCORES)),
                               trace=_trace)
    out = np.concatenate([r["OUT"] for r in res.results], axis=0)
    if _trace:
        kernel.last_exec_time_ns = res.exec_time_ns
        kernel.last_trace = res.instructions_and_trace
    return out


if __name__ == "__main__":
    # quick solver self-check
    rng = np.random.default_rng(0)
    A = rng.standard_normal((5, 4, 4)).astype(np.float32)
    M = np.einsum("bij,bkj->bik", A, A) + 4 * np.eye(4, dtype=np.float32)
    cv = rng.standard_normal((5, 4)).astype(np.float32)
    G = np.zeros((5, 14), dtype=np.float32)
    order = [(0, 0), (1, 0), (1, 1), (2, 0), (2, 1), (2, 2), (3, 0), (3, 1), (3, 2), (3, 3)]
    for ix, (k, j) in enumerate(order):
        G[:, ix] = M[:, k, j]
    G[:, 10:] = cv
    y = _solve_sym4_np(G)
    yref = np.stack([np.linalg.solve(M[i], cv[i]) for i in range(5)])
    print("solver max err:", np.abs(y - yref).max())



# revision 7
# speedup vs baseline: 1.6882x; 1.6882x over previous
"""XL-BOMD rank-4 Krylov propagation (EnergyXL) on 8 TRN2 NeuronCores.

Data-parallel over molecules: 512 mols -> 64 per core.  The operator
A(v) = R v R - v is self-adjoint w.r.t. the Frobenius inner product, so
the reference's full Gram-Schmidt chain collapses to an (unnormalized)
Lanczos 3-term recurrence:

  p_0 = D - P,  n_k = <p_k,p_k>
  W'  = R p_k R              (bf16 PE sandwiches, fp32 PSUM accum)
  a'_k = <W', p_k> / n_k,  b_k = n_k / n_{k-1}
  p_{k+1} = W' - a'_k p_k - b_k p_{k-1}

The final dP2dt2 = -V (W^T W)^-1 W^T dDS is basis-invariant over the
Krylov subspace; O/c are reconstructed from the tridiagonal scalars
(a_k = a'_k - 1, beta_k = sqrt(n_k/n_{k-1})); rank 3 needs no p_4:
O_33 = (<W',W'> - 2 S_3 + n_3)/n_3.  Batched symmetric 4x4 solve per
8-mol block, then out = -sum_k y_k/sqrt(n_k) p_k.

Layout: each 192x192 matrix lives as one [96, 384] SBUF tile (rows
0:96 -> cols 0:192, rows 96:192 -> cols 192:384), so every elementwise
op / reduction is a single instruction.  Matmuls split into 4 [96,96]
weight chunks x 192-wide moving ops into a single [96,384] PSUM bank.
Vectors are stored bf16 (validated 2.6e-3 rel err vs 2e-2 budget);
all reductions accumulate fp32.
"""

import sys

sys.path.insert(0, "/opt/trn_rl_repo")

import numpy as np

import concourse.bass as bass
import concourse.bacc as bacc
import concourse.tile as tile
from concourse import mybir
from concourse.bass_utils import run_bass_kernel_spmd

F32 = mybir.dt.float32
BF16 = mybir.dt.bfloat16
ALU = mybir.AluOpType
ACTF = mybir.ActivationFunctionType

NMOL, N, RANK = 512, 192, 4
NCORES = 8
MPC = NMOL // NCORES  # 64 molecules per core
HP = 96               # partitions per tile (192 rows in 2 col groups)
FW = 384              # free width: 2 x 192
BLK = 8               # molecules per solve block

# partials cols (per mol, [96, 9]): n_k at 2k (k=0..3), S_k at 2k+1, WW at 8
NPART = 9


def build_core_kernel(n_mols=MPC):
    nc = bacc.Bacc(None, target_bir_lowering=False, enable_partition_id=False)
    D = nc.dram_tensor("D", [n_mols, N, N], F32, kind="ExternalInput")
    P = nc.dram_tensor("P", [n_mols, N, N], F32, kind="ExternalInput")
    R = nc.dram_tensor("Rm", [n_mols, N, N], F32, kind="ExternalInput")
    OUT = nc.dram_tensor("OUT", [n_mols, N, N], F32, kind="ExternalOutput")

    with tile.TileContext(nc) as tc:
        _body(nc, tc, D, P, R, OUT)
    nc.finalize()
    return nc


def _load_e(nc, t, X, m):
    """DMA X[m] (192x192 DRAM) into E-layout tile t [96, 384]."""
    nc.sync.dma_start(out=t[:, 0:N], in_=X[m, 0:HP, :])
    nc.sync.dma_start(out=t[:, N:FW], in_=X[m, HP:N, :])


def _store_e(nc, X, m, t):
    nc.sync.dma_start(out=X[m, 0:HP, :], in_=t[:, 0:N])
    nc.sync.dma_start(out=X[m, HP:N, :], in_=t[:, N:FW])


def _sandwich(nc, ps, L, B):
    """ps[96,384] (PSUM) = (L @ B) in E-layout; L symmetric, both bf16.

    out rows 0:96 -> ps[:,0:192], rows 96:192 -> ps[:,192:384];
    contraction split over row groups 0:96 / 96:192.
    """
    mm = nc.tensor.matmul
    mm(ps[:, 0:N], lhsT=L[:, 0:HP], rhs=B[:, 0:N], start=True, stop=False)
    mm(ps[:, 0:N], lhsT=L[:, 2 * HP:3 * HP], rhs=B[:, N:FW], start=False, stop=True)
    mm(ps[:, N:FW], lhsT=L[:, HP:2 * HP], rhs=B[:, 0:N], start=True, stop=False)
    mm(ps[:, N:FW], lhsT=L[:, 3 * HP:FW], rhs=B[:, N:FW], start=False, stop=True)


def _body(nc, tc, D, P, R, OUT):
    import contextlib

    ctx = contextlib.ExitStack()
    with ctx:
        consts = ctx.enter_context(tc.tile_pool(name="consts", bufs=1))
        stage = ctx.enter_context(tc.tile_pool(name="stage", bufs=3))
        pvec = ctx.enter_context(tc.tile_pool(name="pvec", bufs=10))
        work = ctx.enter_context(tc.tile_pool(name="work", bufs=3))
        scal = ctx.enter_context(tc.tile_pool(name="scal", bufs=6))
        blkp = ctx.enter_context(tc.tile_pool(name="blkp", bufs=2))
        ps_T = ctx.enter_context(tc.tile_pool(name="ps_T", bufs=2, space="PSUM"))
        ps_W = ctx.enter_context(tc.tile_pool(name="ps_W", bufs=2, space="PSUM"))
        ps_s = ctx.enter_context(tc.tile_pool(name="ps_s", bufs=1, space="PSUM"))

        # --- constants ---
        ones = consts.tile([HP, HP], F32)
        nc.vector.memset(ones, 1.0)
        sel = consts.tile([HP, 2 * BLK - 1], F32)  # windowed one-hot selector
        nc.vector.memset(sel, 0.0)
        nc.vector.memset(sel[:, BLK - 1:BLK], 1.0)
        id8 = consts.tile([BLK, BLK], F32)
        idt = consts.tile([BLK, BLK], mybir.dt.int32)
        nc.gpsimd.iota(idt, pattern=[[-1, BLK]], base=0, channel_multiplier=1)
        nc.vector.tensor_scalar(out=id8, in0=idt, scalar1=0, scalar2=None,
                                op0=ALU.is_equal)

        n_mols = D.shape[0]
        for b in range(n_mols // BLK):
            mols = list(range(b * BLK, (b + 1) * BLK))
            blk_state = []
            for m in mols:
                st = _mol_pipeline(nc, tc, D, P, R, m, stage, pvec, work, scal,
                                   ps_T, ps_W, ps_s, ones)
                blk_state.append(st)
            _block_tail(nc, tc, OUT, mols, blk_state, work, scal, blkp,
                        ps_s, ones, sel, id8)


def _mol_pipeline(nc, tc, D, P, R, m, stage, pvec, work, scal, ps_T, ps_W,
                  ps_s, ones):
    # --- load + dds = D - P (bf16), n0, R cast ---
    d_st = stage.tile([HP, FW], F32, tag="d_st")
    p_st = stage.tile([HP, FW], F32, tag="p_st")
    r_st = stage.tile([HP, FW], F32, tag="r_st")
    _load_e(nc, d_st, D, m)
    _load_e(nc, p_st, P, m)
    _load_e(nc, r_st, R, m)

    r_bf = work.tile([HP, FW], BF16, tag="r_bf")
    nc.scalar.copy(r_bf, r_st)

    partials = scal.tile([HP, NPART], F32, tag="partials", bufs=10)
    p_tiles = [pvec.tile([HP, FW], BF16, tag=f"p{k}", name=f"p{k}")
               for k in range(RANK)]

    nc.gpsimd.tensor_sub(p_tiles[0], d_st, p_st)
    scr = work.tile([HP, FW], BF16, tag="scr", bufs=4)
    nc.vector.scalar_tensor_tensor(out=scr, in0=p_tiles[0], scalar=1.0,
                                   in1=p_tiles[0], op0=ALU.bypass,
                                   op1=ALU.mult, accum_out=partials[:, 0:1])

    rn_prev = None
    for k in range(RANK):
        pk = p_tiles[k]
        # T = p_k R   ->  W' = R T (E-layout PSUM)
        t_ps = ps_T.tile([HP, FW], F32, tag="t_ps")
        _sandwich(nc, t_ps, pk, r_bf)
        t_bf = work.tile([HP, FW], BF16, tag="t_bf")
        nc.scalar.copy(t_bf, t_ps)
        w_ps = ps_W.tile([HP, FW], F32, tag="w_ps")
        _sandwich(nc, w_ps, r_bf, t_bf)

        # S_k = <W', p_k>  (fused mult+reduce on DVE)
        scr2 = work.tile([HP, FW], BF16, tag="scr", bufs=4)
        nc.vector.scalar_tensor_tensor(out=scr2, in0=pk, scalar=1.0, in1=w_ps,
                                       op0=ALU.bypass, op1=ALU.mult,
                                       accum_out=partials[:, 2 * k + 1:2 * k + 2])

        if k == RANK - 1:
            # last rank: only <W',W'> is needed (O_33 identity), no p_4
            scr3 = work.tile([HP, FW], BF16, tag="scr", bufs=4)
            nc.scalar.activation(out=scr3, in_=w_ps, func=ACTF.Square,
                                 accum_out=partials[:, 8:9])
            break

        # broadcast [n_k, S_k] across partitions via ones-matmul
        bc = ps_s.tile([HP, 4], F32, tag="bc", bufs=2)
        nc.tensor.matmul(bc[:, 0:2], lhsT=ones,
                         rhs=partials[:, 2 * k:2 * k + 2], start=True, stop=True)
        rn = scal.tile([HP, 1], F32, tag="rn", bufs=4)
        nc.vector.reciprocal(rn, bc[:, 0:1])
        na = scal.tile([HP, 1], F32, tag="na", bufs=4)
        nc.vector.tensor_scalar(out=na, in0=bc[:, 1:2], scalar1=rn,
                                scalar2=-1.0, op0=ALU.mult, op1=ALU.mult)

        # p_{k+1} = W' - a'_k p_k - b_k p_{k-1}
        if k == 0:
            nc.vector.scalar_tensor_tensor(out=p_tiles[1], in0=pk, scalar=na,
                                           in1=w_ps, op0=ALU.mult, op1=ALU.add)
        else:
            nb = scal.tile([HP, 1], F32, tag="nb", bufs=4)
            nc.vector.tensor_scalar(out=nb, in0=bc[:, 0:1], scalar1=rn_prev,
                                    scalar2=-1.0, op0=ALU.mult, op1=ALU.mult)
            u1 = work.tile([HP, FW], F32, tag="u1")
            nc.vector.scalar_tensor_tensor(out=u1, in0=pk, scalar=na,
                                           in1=w_ps, op0=ALU.mult, op1=ALU.add)
            t2 = work.tile([HP, FW], BF16, tag="t2")
            nc.gpsimd.tensor_scalar(out=t2, in0=p_tiles[k - 1], scalar1=nb,
                                    scalar2=None, op0=ALU.mult)
            nc.gpsimd.tensor_add(p_tiles[k + 1], u1, t2)
        rn_prev = rn

        # n_{k+1} = <p_{k+1}, p_{k+1}>
        scr4 = work.tile([HP, FW], BF16, tag="scr", bufs=4)
        nc.vector.scalar_tensor_tensor(out=scr4, in0=p_tiles[k + 1], scalar=1.0,
                                       in1=p_tiles[k + 1], op0=ALU.bypass,
                                       op1=ALU.mult,
                                       accum_out=partials[:, 2 * k + 2:2 * k + 3])

    return {"partials": partials, "p": p_tiles}


def _solve_sym4(nc, g, s):
    """Batched symmetric 4x4 solve on [BLK,1] column APs.

    g: [BLK, 14] tile, cols 0..9 = O (00,10,11,20,21,22,30,31,32,33),
    cols 10..13 = rhs c.  s: [BLK, 16] scratch.  Returns y col APs.
    """
    def col(t, i):
        return t[:, i:i + 1]

    a, bb, e, c, f, h, d, gg, i_, jj = (col(g, i) for i in range(10))
    r0, r1, r2, r3 = (col(g, 10 + i) for i in range(4))
    p0, p1, p2, p3 = (col(s, 4 + i) for i in range(4))
    l1, l2, l3 = (col(s, 8 + i) for i in range(3))
    t0 = col(s, 11)
    y0, y1, y2, y3 = (col(s, i) for i in range(4))

    mul = nc.vector.tensor_mul
    sub = nc.vector.tensor_sub
    rec = nc.vector.reciprocal

    def upd(x, l, src):  # x -= l*src
        mul(t0, l, src)
        sub(x, x, t0)

    rec(p0, a)
    mul(l1, bb, p0); mul(l2, c, p0); mul(l3, d, p0)
    upd(e, l1, bb); upd(f, l2, bb); upd(gg, l3, bb)
    upd(h, l2, c); upd(i_, l3, c); upd(jj, l3, d)
    upd(r1, l1, r0); upd(r2, l2, r0); upd(r3, l3, r0)

    rec(p1, e)
    mul(l2, f, p1); mul(l3, gg, p1)
    upd(h, l2, f); upd(i_, l3, f); upd(jj, l3, gg)
    upd(r2, l2, r1); upd(r3, l3, r1)

    rec(p2, h)
    mul(l3, i_, p2)
    upd(jj, l3, i_); upd(r3, l3, r2)

    rec(p3, jj)
    mul(y3, r3, p3)
    upd(r2, i_, y3); mul(y2, r2, p2)
    upd(r1, f, y2); upd(r1, gg, y3); mul(y1, r1, p1)
    upd(r0, bb, y1); upd(r0, c, y2); upd(r0, d, y3); mul(y0, r0, p0)
    return [y0, y1, y2, y3]


def _block_tail(nc, tc, OUT, mols, blk_state, work, scal, blkp, ps_s, ones,
                sel, id8):
    # gather each mol's 9 partial sums into [BLK, 9] rows via selector matmuls
    gath = ps_s.tile([BLK, NPART], F32, tag="gath")
    for j, st in enumerate(blk_state):
        nc.tensor.matmul(gath, lhsT=sel[:, BLK - 1 - j:2 * BLK - 1 - j],
                         rhs=st["partials"][:, 0:NPART],
                         start=(j == 0), stop=(j == len(blk_state) - 1))
    gb = blkp.tile([BLK, NPART], F32, tag="gb")
    nc.scalar.copy(gb, gath)

    # tridiagonal scalars -> O (10 cols) + c (4 cols)
    w = blkp.tile([BLK, 40], F32, tag="w")
    nv = gb[:, 0:8:2]                          # [8,4] n_0..n_3
    sv = gb[:, 1:8:2]                          # [8,4] S_0..S_3
    ww = gb[:, 8:9]                            # [8,1] <W'_3, W'_3>
    rn4 = w[:, 0:4]
    sq4 = w[:, 4:8]
    rsq4 = w[:, 8:12]
    av = w[:, 12:16]
    bv = w[:, 16:19]                           # beta_1..beta_3
    asq = w[:, 19:23]
    bz = w[:, 23:27]                           # [8,4]: 0, b1^2, b2^2, b3^2
    odg = w[:, 27:30]                          # O_00..O_22
    t3 = w[:, 30:33]
    o33 = w[:, 33:34]
    nc.vector.reciprocal(rn4, nv)
    nc.scalar.sqrt(sq4, nv)
    nc.vector.reciprocal(rsq4, sq4)
    nc.vector.tensor_mul(av, sv, rn4)                    # a' = S/n
    nc.vector.tensor_scalar(out=av, in0=av, scalar1=1.0, scalar2=None,
                            op0=ALU.subtract)            # a = a' - 1
    nc.vector.tensor_mul(bv, sq4[:, 1:4], rsq4[:, 0:3])  # beta_{k+1}
    nc.vector.tensor_mul(asq, av, av)
    nc.vector.memset(bz[:, 0:1], 0.0)
    nc.vector.tensor_mul(bz[:, 1:4], bv, bv)
    nc.vector.tensor_add(odg, asq[:, 0:3], bz[:, 0:3])
    nc.vector.tensor_add(odg, odg, bz[:, 1:4])           # O_kk, k=0..2
    # O_33 = (ww - 2 S_3 + n_3) / n_3
    nc.vector.tensor_add(o33, ww, nv[:, 3:4])
    nc.vector.tensor_scalar(out=w[:, 34:35], in0=sv[:, 3:4], scalar1=-2.0,
                            scalar2=None, op0=ALU.mult)
    nc.vector.tensor_add(o33, o33, w[:, 34:35])
    nc.vector.tensor_mul(o33, o33, rn4[:, 3:4])
    nc.vector.tensor_add(t3, av[:, 0:3], av[:, 1:4])
    nc.vector.tensor_mul(t3, t3, bv)                     # O_{k,k+1}

    g = blkp.tile([BLK, 14], F32, tag="g")
    # diag -> cols 0,2,5,9 ; off1 -> 1,4,8 ; off2 -> 3,7 ; O30 -> 6
    for i, cdst in enumerate((0, 2, 5)):
        nc.vector.tensor_copy(g[:, cdst:cdst + 1], odg[:, i:i + 1])
    nc.vector.tensor_copy(g[:, 9:10], o33)
    for i, cdst in enumerate((1, 4, 8)):
        nc.vector.tensor_copy(g[:, cdst:cdst + 1], t3[:, i:i + 1])
    nc.vector.tensor_mul(g[:, 3:4], bv[:, 0:1], bv[:, 1:2])
    nc.vector.tensor_mul(g[:, 7:8], bv[:, 1:2], bv[:, 2:3])
    nc.vector.memset(g[:, 6:7], 0.0)
    nc.vector.tensor_mul(g[:, 10:11], av[:, 0:1], sq4[:, 0:1])  # c0
    nc.vector.tensor_copy(g[:, 11:12], sq4[:, 1:2])             # c1
    nc.vector.memset(g[:, 12:14], 0.0)

    s_sb = blkp.tile([BLK, 16], F32, tag="s_sb")
    ys = _solve_sym4(nc, g, s_sb)
    yneg = blkp.tile([BLK, RANK], F32, tag="yneg")
    for i in range(RANK):
        nc.vector.tensor_copy(yneg[:, i:i + 1], ys[i])
    nc.vector.tensor_mul(yneg, yneg, rsq4)
    nc.vector.tensor_scalar(out=yneg, in0=yneg, scalar1=-1.0, scalar2=None,
                            op0=ALU.mult)

    for j, (m, st) in enumerate(zip(mols, blk_state)):
        ymask = scal.tile([BLK, RANK], F32, tag="ymask")
        nc.vector.tensor_scalar(out=ymask, in0=yneg, scalar1=id8[:, j:j + 1],
                                scalar2=None, op0=ALU.mult)
        ybc = ps_s.tile([HP, RANK], F32, tag="ybc")
        nc.tensor.matmul(ybc, lhsT=ones[0:BLK, :], rhs=ymask, start=True,
                         stop=True)
        yb = scal.tile([HP, RANK], F32, tag="yb")
        nc.scalar.copy(yb, ybc)

        p = st["p"]
        acc = work.tile([HP, FW], F32, tag="acc")
        nc.gpsimd.tensor_scalar(out=acc, in0=p[0], scalar1=yb[:, 0:1],
                                scalar2=None, op0=ALU.mult)
        acc1 = work.tile([HP, FW], F32, tag="acc1")
        nc.vector.scalar_tensor_tensor(out=acc1, in0=p[1], scalar=yb[:, 1:2],
                                       in1=acc, op0=ALU.mult, op1=ALU.add)
        t2c = work.tile([HP, FW], F32, tag="t2c")
        nc.gpsimd.tensor_scalar(out=t2c, in0=p[2], scalar1=yb[:, 2:3],
                                scalar2=None, op0=ALU.mult)
        acc2 = work.tile([HP, FW], F32, tag="acc2")
        nc.gpsimd.tensor_add(acc2, acc1, t2c)
        acc3 = work.tile([HP, FW], F32, tag="acc3")
        nc.vector.scalar_tensor_tensor(out=acc3, in0=p[3], scalar=yb[:, 3:4],
                                       in1=acc2, op0=ALU.mult, op1=ALU.add)
        _store_e(nc, OUT, m, acc3)


_NC_CACHE = None


def _get_nc():
    global _NC_CACHE
    if _NC_CACHE is None:
        _NC_CACHE = build_core_kernel()
    return _NC_CACHE


def kernel(D, P, R, max_rank=4, _trace=False):
    D = np.ascontiguousarray(D, dtype=np.float32)
    P = np.ascontiguousarray(P, dtype=np.float32)
    R = np.ascontiguousarray(R, dtype=np.float32)
    nc = _get_nc()
    in_maps = []
    for i in range(NCORES):
        sl = slice(i * MPC, (i + 1) * MPC)
        in_maps.append({"D": D[sl], "P": P[sl], "Rm": R[sl]})
    res = run_bass_kernel_spmd(nc, in_maps, core_ids=list(range(NCORES)),
                               trace=_trace)
    out = np.concatenate([r["OUT"] for r in res.results], axis=0)
    if _trace:
        kernel.last_exec_time_ns = res.exec_time_ns
        kernel.last_trace = res.instructions_and_trace
    return out


# revision 8
# speedup vs baseline: 3.3852x; 2.0052x over previous
"""XL-BOMD rank-4 Krylov propagation (EnergyXL) on 8 TRN2 NeuronCores.

Data-parallel over molecules: 512 mols -> 64 per core.  The operator
A(v) = R v R - v is self-adjoint w.r.t. the Frobenius inner product, so
the reference's full Gram-Schmidt chain collapses to an (unnormalized)
Lanczos 3-term recurrence:

  p_0 = D - P,  n_k = <p_k,p_k>
  W'  = R p_k R              (bf16 PE sandwiches, fp32 PSUM accum)
  a'_k = <W', p_k> / n_k,  b_k = n_k / n_{k-1}
  p_{k+1} = W' - a'_k p_k - b_k p_{k-1}

The final dP2dt2 = -V (W^T W)^-1 W^T dDS is basis-invariant over the
Krylov subspace; O/c are reconstructed from the tridiagonal scalars
(a_k = a'_k - 1, beta_k = sqrt(n_k/n_{k-1})); rank 3 needs no p_4:
O_33 = (<W',W'> - 2 S_3 + n_3)/n_3.  Batched symmetric 4x4 solve per
8-mol block, then out = -sum_k y_k/sqrt(n_k) p_k.

Layout: each 192x192 matrix lives as one [96, 384] SBUF tile (rows
0:96 -> cols 0:192, rows 96:192 -> cols 192:384), so every elementwise
op / reduction is a single instruction.  Matmuls split into 4 [96,96]
weight chunks x 192-wide moving ops into a single [96,384] PSUM bank.
Vectors are stored bf16 (validated 2.6e-3 rel err vs 2e-2 budget);
all reductions accumulate fp32.
"""

import sys

sys.path.insert(0, "/opt/trn_rl_repo")

import numpy as np

import concourse.bass as bass
import concourse.bacc as bacc
import concourse.tile as tile
from concourse import mybir
from concourse.bass_utils import run_bass_kernel_spmd

F32 = mybir.dt.float32
BF16 = mybir.dt.bfloat16
ALU = mybir.AluOpType
ACTF = mybir.ActivationFunctionType

NMOL, N, RANK = 512, 192, 4
NCORES = 8
MPC = NMOL // NCORES  # 64 molecules per core
HP = 96               # partitions per tile (192 rows in 2 col groups)
FW = 384              # free width: 2 x 192
BLK = 8               # molecules per solve block

# partials cols (per mol, [96, 9]): n_k at 2k (k=0..3), S_k at 2k+1, WW at 8
NPART = 9


def build_core_kernel(n_mols=MPC):
    nc = bacc.Bacc(None, target_bir_lowering=False, enable_partition_id=False)
    D = nc.dram_tensor("D", [n_mols, N, N], F32, kind="ExternalInput")
    P = nc.dram_tensor("P", [n_mols, N, N], F32, kind="ExternalInput")
    R = nc.dram_tensor("Rm", [n_mols, N, N], F32, kind="ExternalInput")
    OUT = nc.dram_tensor("OUT", [n_mols, N, N], F32, kind="ExternalOutput")

    with tile.TileContext(nc) as tc:
        _body(nc, tc, D, P, R, OUT)
    nc.finalize()
    return nc


def _load_e(nc, t, X, m):
    """DMA X[m] (192x192 DRAM) into E-layout tile t [96, 384]."""
    nc.sync.dma_start(out=t[:, 0:N], in_=X[m, 0:HP, :])
    nc.sync.dma_start(out=t[:, N:FW], in_=X[m, HP:N, :])


def _store_e(nc, X, m, t):
    nc.sync.dma_start(out=X[m, 0:HP, :], in_=t[:, 0:N])
    nc.sync.dma_start(out=X[m, HP:N, :], in_=t[:, N:FW])


def _sandwich(nc, ps, L, B):
    """ps[96,384] (PSUM) = (L @ B) in E-layout; L symmetric, both bf16.

    out rows 0:96 -> ps[:,0:192], rows 96:192 -> ps[:,192:384];
    contraction split over row groups 0:96 / 96:192.
    """
    mm = nc.tensor.matmul
    mm(ps[:, 0:N], lhsT=L[:, 0:HP], rhs=B[:, 0:N], start=True, stop=False)
    mm(ps[:, 0:N], lhsT=L[:, 2 * HP:3 * HP], rhs=B[:, N:FW], start=False, stop=True)
    mm(ps[:, N:FW], lhsT=L[:, HP:2 * HP], rhs=B[:, 0:N], start=True, stop=False)
    mm(ps[:, N:FW], lhsT=L[:, 3 * HP:FW], rhs=B[:, N:FW], start=False, stop=True)


def _body(nc, tc, D, P, R, OUT):
    import contextlib

    ctx = contextlib.ExitStack()
    with ctx:
        consts = ctx.enter_context(tc.tile_pool(name="consts", bufs=1))
        stage = ctx.enter_context(tc.tile_pool(name="stage", bufs=3))
        pvec = ctx.enter_context(tc.tile_pool(name="pvec", bufs=10))
        work = ctx.enter_context(tc.tile_pool(name="work", bufs=3))
        scal = ctx.enter_context(tc.tile_pool(name="scal", bufs=6))
        blkp = ctx.enter_context(tc.tile_pool(name="blkp", bufs=2))
        ps_T = ctx.enter_context(tc.tile_pool(name="ps_T", bufs=2, space="PSUM"))
        ps_W = ctx.enter_context(tc.tile_pool(name="ps_W", bufs=2, space="PSUM"))
        ps_s = ctx.enter_context(tc.tile_pool(name="ps_s", bufs=1, space="PSUM"))

        # --- constants ---
        ones = consts.tile([HP, HP], F32)
        nc.vector.memset(ones, 1.0)
        sel = consts.tile([HP, 2 * BLK - 1], F32)  # windowed one-hot selector
        nc.vector.memset(sel, 0.0)
        nc.vector.memset(sel[:, BLK - 1:BLK], 1.0)
        id8 = consts.tile([BLK, BLK], F32)
        idt = consts.tile([BLK, BLK], mybir.dt.int32)
        nc.gpsimd.iota(idt, pattern=[[-1, BLK]], base=0, channel_multiplier=1)
        nc.vector.tensor_scalar(out=id8, in0=idt, scalar1=0, scalar2=None,
                                op0=ALU.is_equal)

        n_mols = D.shape[0]
        for b in range(n_mols // BLK):
            mols = list(range(b * BLK, (b + 1) * BLK))
            blk_state = []
            for m in mols:
                st = _mol_pipeline(nc, tc, D, P, R, m, stage, pvec, work, scal,
                                   ps_T, ps_W, ps_s, ones)
                blk_state.append(st)
            _block_tail(nc, tc, OUT, mols, blk_state, work, scal, blkp,
                        ps_s, ones, sel, id8)


def _mol_pipeline(nc, tc, D, P, R, m, stage, pvec, work, scal, ps_T, ps_W,
                  ps_s, ones):
    # --- load + dds = D - P (bf16), n0, R cast ---
    d_st = stage.tile([HP, FW], F32, tag="d_st")
    p_st = stage.tile([HP, FW], F32, tag="p_st")
    r_st = stage.tile([HP, FW], F32, tag="r_st")
    _load_e(nc, d_st, D, m)
    _load_e(nc, p_st, P, m)
    _load_e(nc, r_st, R, m)

    r_bf = work.tile([HP, FW], BF16, tag="r_bf")
    nc.scalar.copy(r_bf, r_st)

    partials = scal.tile([HP, NPART], F32, tag="partials", bufs=10)
    p_tiles = [pvec.tile([HP, FW], BF16, tag=f"p{k}", name=f"p{k}")
               for k in range(RANK)]

    nc.vector.tensor_sub(p_tiles[0], d_st, p_st)
    scr = work.tile([HP, FW], BF16, tag="scr", bufs=4)
    nc.scalar.activation(out=scr, in_=p_tiles[0], func=ACTF.Square,
                         accum_out=partials[:, 0:1])

    rn_prev = None
    for k in range(RANK):
        pk = p_tiles[k]
        # T = p_k R   ->  W' = R T (E-layout PSUM)
        t_ps = ps_T.tile([HP, FW], F32, tag="t_ps")
        _sandwich(nc, t_ps, pk, r_bf)
        t_bf = work.tile([HP, FW], BF16, tag="t_bf")
        nc.scalar.copy(t_bf, t_ps)
        w_ps = ps_W.tile([HP, FW], F32, tag="w_ps")
        _sandwich(nc, w_ps, r_bf, t_bf)

        # S_k = <W', p_k>  (fused mult+reduce on DVE)
        scr2 = work.tile([HP, FW], BF16, tag="scr", bufs=4)
        nc.vector.scalar_tensor_tensor(out=scr2, in0=pk, scalar=1.0, in1=w_ps,
                                       op0=ALU.bypass, op1=ALU.mult,
                                       accum_out=partials[:, 2 * k + 1:2 * k + 2])

        if k == RANK - 1:
            # last rank: only <W',W'> is needed (O_33 identity), no p_4
            scr3 = work.tile([HP, FW], BF16, tag="scr", bufs=4)
            nc.scalar.activation(out=scr3, in_=w_ps, func=ACTF.Square,
                                 accum_out=partials[:, 8:9])
            break

        # broadcast [n_k, S_k] across partitions via ones-matmul
        bc = ps_s.tile([HP, 4], F32, tag="bc", bufs=2)
        nc.tensor.matmul(bc[:, 0:2], lhsT=ones,
                         rhs=partials[:, 2 * k:2 * k + 2], start=True, stop=True)
        rn = scal.tile([HP, 1], F32, tag="rn", bufs=4)
        nc.vector.reciprocal(rn, bc[:, 0:1])
        na = scal.tile([HP, 1], F32, tag="na", bufs=4)
        nc.vector.tensor_scalar(out=na, in0=bc[:, 1:2], scalar1=rn,
                                scalar2=-1.0, op0=ALU.mult, op1=ALU.mult)

        # p_{k+1} = W' - a'_k p_k - b_k p_{k-1}
        if k == 0:
            nc.vector.scalar_tensor_tensor(out=p_tiles[1], in0=pk, scalar=na,
                                           in1=w_ps, op0=ALU.mult, op1=ALU.add)
        else:
            nb = scal.tile([HP, 1], F32, tag="nb", bufs=4)
            nc.vector.tensor_scalar(out=nb, in0=bc[:, 0:1], scalar1=rn_prev,
                                    scalar2=-1.0, op0=ALU.mult, op1=ALU.mult)
            u1 = work.tile([HP, FW], F32, tag="u1")
            nc.vector.scalar_tensor_tensor(out=u1, in0=pk, scalar=na,
                                           in1=w_ps, op0=ALU.mult, op1=ALU.add)
            t2 = work.tile([HP, FW], BF16, tag="t2")
            nc.scalar.activation(out=t2, in_=p_tiles[k - 1], func=ACTF.Copy,
                                 scale=nb)
            nc.vector.tensor_add(p_tiles[k + 1], u1, t2)
        rn_prev = rn

        # n_{k+1} = <p_{k+1}, p_{k+1}>
        scr4 = work.tile([HP, FW], BF16, tag="scr", bufs=4)
        nc.scalar.activation(out=scr4, in_=p_tiles[k + 1], func=ACTF.Square,
                             accum_out=partials[:, 2 * k + 2:2 * k + 3])

    return {"partials": partials, "p": p_tiles}


def _solve_sym4(nc, g, s):
    """Batched symmetric 4x4 solve on [BLK,1] column APs.

    g: [BLK, 14] tile, cols 0..9 = O (00,10,11,20,21,22,30,31,32,33),
    cols 10..13 = rhs c.  s: [BLK, 16] scratch.  Returns y col APs.
    """
    def col(t, i):
        return t[:, i:i + 1]

    a, bb, e, c, f, h, d, gg, i_, jj = (col(g, i) for i in range(10))
    r0, r1, r2, r3 = (col(g, 10 + i) for i in range(4))
    p0, p1, p2, p3 = (col(s, 4 + i) for i in range(4))
    l1, l2, l3 = (col(s, 8 + i) for i in range(3))
    t0 = col(s, 11)
    y0, y1, y2, y3 = (col(s, i) for i in range(4))

    mul = nc.vector.tensor_mul
    sub = nc.vector.tensor_sub
    rec = nc.vector.reciprocal

    def upd(x, l, src):  # x -= l*src
        mul(t0, l, src)
        sub(x, x, t0)

    rec(p0, a)
    mul(l1, bb, p0); mul(l2, c, p0); mul(l3, d, p0)
    upd(e, l1, bb); upd(f, l2, bb); upd(gg, l3, bb)
    upd(h, l2, c); upd(i_, l3, c); upd(jj, l3, d)
    upd(r1, l1, r0); upd(r2, l2, r0); upd(r3, l3, r0)

    rec(p1, e)
    mul(l2, f, p1); mul(l3, gg, p1)
    upd(h, l2, f); upd(i_, l3, f); upd(jj, l3, gg)
    upd(r2, l2, r1); upd(r3, l3, r1)

    rec(p2, h)
    mul(l3, i_, p2)
    upd(jj, l3, i_); upd(r3, l3, r2)

    rec(p3, jj)
    mul(y3, r3, p3)
    upd(r2, i_, y3); mul(y2, r2, p2)
    upd(r1, f, y2); upd(r1, gg, y3); mul(y1, r1, p1)
    upd(r0, bb, y1); upd(r0, c, y2); upd(r0, d, y3); mul(y0, r0, p0)
    return [y0, y1, y2, y3]


def _block_tail(nc, tc, OUT, mols, blk_state, work, scal, blkp, ps_s, ones,
                sel, id8):
    # gather each mol's 9 partial sums into [BLK, 9] rows via selector matmuls
    gath = ps_s.tile([BLK, NPART], F32, tag="gath")
    for j, st in enumerate(blk_state):
        nc.tensor.matmul(gath, lhsT=sel[:, BLK - 1 - j:2 * BLK - 1 - j],
                         rhs=st["partials"][:, 0:NPART],
                         start=(j == 0), stop=(j == len(blk_state) - 1))
    gb = blkp.tile([BLK, NPART], F32, tag="gb")
    nc.scalar.copy(gb, gath)

    # tridiagonal scalars -> O (10 cols) + c (4 cols)
    w = blkp.tile([BLK, 40], F32, tag="w")
    nv = gb[:, 0:8:2]                          # [8,4] n_0..n_3
    sv = gb[:, 1:8:2]                          # [8,4] S_0..S_3
    ww = gb[:, 8:9]                            # [8,1] <W'_3, W'_3>
    rn4 = w[:, 0:4]
    sq4 = w[:, 4:8]
    rsq4 = w[:, 8:12]
    av = w[:, 12:16]
    bv = w[:, 16:19]                           # beta_1..beta_3
    asq = w[:, 19:23]
    bz = w[:, 23:27]                           # [8,4]: 0, b1^2, b2^2, b3^2
    odg = w[:, 27:30]                          # O_00..O_22
    t3 = w[:, 30:33]
    o33 = w[:, 33:34]
    nc.vector.reciprocal(rn4, nv)
    nc.scalar.sqrt(sq4, nv)
    nc.vector.reciprocal(rsq4, sq4)
    nc.vector.tensor_mul(av, sv, rn4)                    # a' = S/n
    nc.vector.tensor_scalar(out=av, in0=av, scalar1=1.0, scalar2=None,
                            op0=ALU.subtract)            # a = a' - 1
    nc.vector.tensor_mul(bv, sq4[:, 1:4], rsq4[:, 0:3])  # beta_{k+1}
    nc.vector.tensor_mul(asq, av, av)
    nc.vector.memset(bz[:, 0:1], 0.0)
    nc.vector.tensor_mul(bz[:, 1:4], bv, bv)
    nc.vector.tensor_add(odg, asq[:, 0:3], bz[:, 0:3])
    nc.vector.tensor_add(odg, odg, bz[:, 1:4])           # O_kk, k=0..2
    # O_33 = (ww - 2 S_3 + n_3) / n_3
    nc.vector.tensor_add(o33, ww, nv[:, 3:4])
    nc.vector.tensor_scalar(out=w[:, 34:35], in0=sv[:, 3:4], scalar1=-2.0,
                            scalar2=None, op0=ALU.mult)
    nc.vector.tensor_add(o33, o33, w[:, 34:35])
    nc.vector.tensor_mul(o33, o33, rn4[:, 3:4])
    nc.vector.tensor_add(t3, av[:, 0:3], av[:, 1:4])
    nc.vector.tensor_mul(t3, t3, bv)                     # O_{k,k+1}

    g = blkp.tile([BLK, 14], F32, tag="g")
    # diag -> cols 0,2,5,9 ; off1 -> 1,4,8 ; off2 -> 3,7 ; O30 -> 6
    for i, cdst in enumerate((0, 2, 5)):
        nc.vector.tensor_copy(g[:, cdst:cdst + 1], odg[:, i:i + 1])
    nc.vector.tensor_copy(g[:, 9:10], o33)
    for i, cdst in enumerate((1, 4, 8)):
        nc.vector.tensor_copy(g[:, cdst:cdst + 1], t3[:, i:i + 1])
    nc.vector.tensor_mul(g[:, 3:4], bv[:, 0:1], bv[:, 1:2])
    nc.vector.tensor_mul(g[:, 7:8], bv[:, 1:2], bv[:, 2:3])
    nc.vector.memset(g[:, 6:7], 0.0)
    nc.vector.tensor_mul(g[:, 10:11], av[:, 0:1], sq4[:, 0:1])  # c0
    nc.vector.tensor_copy(g[:, 11:12], sq4[:, 1:2])             # c1
    nc.vector.memset(g[:, 12:14], 0.0)

    s_sb = blkp.tile([BLK, 16], F32, tag="s_sb")
    ys = _solve_sym4(nc, g, s_sb)
    yneg = blkp.tile([BLK, RANK], F32, tag="yneg")
    for i in range(RANK):
        nc.vector.tensor_copy(yneg[:, i:i + 1], ys[i])
    nc.vector.tensor_mul(yneg, yneg, rsq4)
    nc.vector.tensor_scalar(out=yneg, in0=yneg, scalar1=-1.0, scalar2=None,
                            op0=ALU.mult)

    for j, (m, st) in enumerate(zip(mols, blk_state)):
        ymask = scal.tile([BLK, RANK], F32, tag="ymask")
        nc.vector.tensor_scalar(out=ymask, in0=yneg, scalar1=id8[:, j:j + 1],
                                scalar2=None, op0=ALU.mult)
        ybc = ps_s.tile([HP, RANK], F32, tag="ybc")
        nc.tensor.matmul(ybc, lhsT=ones[0:BLK, :], rhs=ymask, start=True,
                         stop=True)
        yb = scal.tile([HP, RANK], F32, tag="yb")
        nc.scalar.copy(yb, ybc)

        p = st["p"]
        acc = work.tile([HP, FW], F32, tag="acc")
        nc.scalar.activation(out=acc, in_=p[0], func=ACTF.Copy,
                             scale=yb[:, 0:1])
        acc1 = work.tile([HP, FW], F32, tag="acc1")
        nc.vector.scalar_tensor_tensor(out=acc1, in0=p[1], scalar=yb[:, 1:2],
                                       in1=acc, op0=ALU.mult, op1=ALU.add)
        t2c = work.tile([HP, FW], F32, tag="t2c")
        nc.scalar.activation(out=t2c, in_=p[2], func=ACTF.Copy,
                             scale=yb[:, 2:3])
        acc2 = work.tile([HP, FW], F32, tag="acc2")
        nc.vector.tensor_add(acc2, acc1, t2c)
        acc3 = work.tile([HP, FW], F32, tag="acc3")
        nc.vector.scalar_tensor_tensor(out=acc3, in0=p[3], scalar=yb[:, 3:4],
                                       in1=acc2, op0=ALU.mult, op1=ALU.add)
        _store_e(nc, OUT, m, acc3)


_NC_CACHE = None


def _get_nc():
    global _NC_CACHE
    if _NC_CACHE is None:
        _NC_CACHE = build_core_kernel()
    return _NC_CACHE


def kernel(D, P, R, max_rank=4, _trace=False):
    D = np.ascontiguousarray(D, dtype=np.float32)
    P = np.ascontiguousarray(P, dtype=np.float32)
    R = np.ascontiguousarray(R, dtype=np.float32)
    nc = _get_nc()
    in_maps = []
    for i in range(NCORES):
        sl = slice(i * MPC, (i + 1) * MPC)
        in_maps.append({"D": D[sl], "P": P[sl], "Rm": R[sl]})
    res = run_bass_kernel_spmd(nc, in_maps, core_ids=list(range(NCORES)),
                               trace=_trace)
    out = np.concatenate([r["OUT"] for r in res.results], axis=0)
    if _trace:
        kernel.last_exec_time_ns = res.exec_time_ns
        kernel.last_trace = res.instructions_and_trace
    return out
